# revision 5
# baseline (speedup 1.0000x reference)
"""Trainium2 Bass kernel for a dense transformer block (B=8, S=512, D=768, H=12, Fd=3072).

Sharding: pure data-parallel over batch — one batch element per NeuronCore,
weights replicated, no collectives.

Math layout trick: activations are kept feature-major ("T layout", [feat, seq])
through the attention pipeline so the TensorEngine (which contracts over the
partition dim) never needs an on-device transpose of the attention matrix:

  xT [768,512] (host-transposed)
  qT = wq.T @ xT, kT = wk.T @ xT          (T layout, per-partition bias fuse)
  v  = xT.T @ wv (natural [t,d] layout, bias via K-augmentation)
  scoresT[t,s] = kT_h[:,tchunk].T @ qT_h  (softmax denominator handled later)
  expT = exp(scoresT + gaussT + maskbias[t])        (no max-subtraction; scores are O(1))
  hT_aug[d+1, s] = [v_h | 1].T @ expT     (extra ones-column gives the softmax sums)
  hT = hT_aug[:64] * broadcast(1/sum)     (broadcast via rank-1 matmul)
  proj = hT_all.T @ wproj (+ bias row)    -> natural [s, 768]; residual + LN1
  h1T  = PE-transpose(h1)                 (24 128x128 transposes)
  ff1T = w1.T @ h1T, gelu fused on PSUM->SBUF copy with per-partition b1
  ff2  = ff1T.T @ w2 (+ bias row)         -> natural; residual + LN2 -> out

Matmul inputs are bf16 (weights pre-cast on host), PSUM accumulation f32,
all element-wise math f32.
"""

import numpy as np
import ml_dtypes

import concourse.bass as bass
import concourse.mybir as mybir
import concourse.tile as tile
from concourse import bacc
from concourse import bass_utils
from concourse.masks import make_identity

BF = mybir.dt.bfloat16
F32 = mybir.dt.float32
AF = mybir.ActivationFunctionType
OP = mybir.AluOpType

B, S, D, H, Dh, Fd = 8, 512, 768, 12, 64, 3072
NCORES = 8
MASK_NEG = -30.0  # effectively -inf after exp given |scores+gauss| <~ 8
EPS = 1e-12

KD = D // 128      # 6  K-tiles over D
MS = S // 128      # 4  chunks over sequence
KF = Fd // 128     # 24 K-tiles over Fd
NT = 2             # N-tiles over D for natural-layout outputs (2 x 384)
ND = D // NT       # 384


def _trace(nc, io):
    tc_ctx = tile.TileContext(nc)
    with tc_ctx as tc:
        _trace_body(nc, tc, io)


def _trace_body(nc, tc, io):
    from contextlib import ExitStack

    with ExitStack() as ctx:
        glob = ctx.enter_context(tc.tile_pool(name="glob", bufs=1))
        psum = ctx.enter_context(tc.tile_pool(name="psum", bufs=1, space="PSUM"))

        # ---- constants / small inputs ----
        ident_bf = glob.tile([128, 128], BF, tag="ident")
        make_identity(nc, ident_bf)
        ones_bf = glob.tile([1, 512], BF, tag="ones_bf")
        nc.vector.memset(ones_bf, 1.0)
        ones64_f = glob.tile([1, 64], F32, tag="ones64")
        nc.vector.memset(ones64_f, 1.0)
        eps_t = glob.tile([128, 1], F32, tag="eps")
        nc.vector.memset(eps_t, EPS)

        bq_c = glob.tile([128, KD], F32, tag="bq")
        nc.sync.dma_start(out=bq_c, in_=io["bq"].rearrange("(c p) -> p c", p=128))
        bk_c = glob.tile([128, KD], F32, tag="bk")
        nc.sync.dma_start(out=bk_c, in_=io["bk"].rearrange("(c p) -> p c", p=128))
        b1_c = glob.tile([128, KF], F32, tag="b1")
        nc.sync.dma_start(out=b1_c, in_=io["b1"].rearrange("(c p) -> p c", p=128))
        maskb_c = glob.tile([128, MS], F32, tag="maskb")
        nc.sync.dma_start(out=maskb_c, in_=io["maskbias"].rearrange("(c p) -> p c", p=128))

        bv_r = glob.tile([1, D], BF, tag="bv")
        nc.sync.dma_start(out=bv_r, in_=io["bv_bf"].rearrange("(a n) -> a n", a=1))
        bproj_r = glob.tile([1, D], BF, tag="bproj")
        nc.sync.dma_start(out=bproj_r, in_=io["bproj_bf"].rearrange("(a n) -> a n", a=1))
        b2_r = glob.tile([1, D], BF, tag="b2")
        nc.sync.dma_start(out=b2_r, in_=io["b2_bf"].rearrange("(a n) -> a n", a=1))

        def bcast128(ap):
            return bass.AP(tensor=ap.tensor, offset=ap.offset, ap=[[0, 128]] + list(ap.ap))

        g1b = glob.tile([128, D], F32, tag="g1b")
        nc.gpsimd.dma_start(out=g1b, in_=bcast128(io["gamma1"]))
        be1b = glob.tile([128, D], F32, tag="be1b")
        nc.gpsimd.dma_start(out=be1b, in_=bcast128(io["beta1"]))
        g2b = glob.tile([128, D], F32, tag="g2b")
        nc.gpsimd.dma_start(out=g2b, in_=bcast128(io["gamma2"]))
        be2b = glob.tile([128, D], F32, tag="be2b")
        nc.gpsimd.dma_start(out=be2b, in_=bcast128(io["beta2"]))

        # ---- big resident activations ----
        x_sb = glob.tile([128, MS, D], F32, tag="x")
        nc.sync.dma_start(out=x_sb, in_=io["x"].rearrange("(c p) n -> p c n", p=128))
        h1_sb = glob.tile([128, MS, D], F32, tag="h1")
        h1T_sb = glob.tile([128, KD, S], BF, tag="h1T")
        ff1T_sb = glob.tile([128, KF, S], BF, tag="ff1T")

        # ================= attention scope =================
        with tc.tile_pool(name="attn", bufs=1) as attnp:
            xT_sb = attnp.tile([128, KD, S], BF, tag="xT")
            nc.sync.dma_start(out=xT_sb, in_=io["xT_bf"].rearrange("(c p) s -> p c s", p=128))
            gauss_sb = attnp.tile([128, MS, S], F32, tag="gauss")
            nc.sync.dma_start(out=gauss_sb, in_=io["gaussT"].rearrange("(c p) s -> p c s", p=128))

            qT_sb = attnp.tile([128, KD, S], BF, tag="qT")
            kT_sb = attnp.tile([128, KD, S], BF, tag="kT")
            v_sb = attnp.tile([128, MS, H, Dh + 1], BF, tag="v")
            nc.vector.memset(v_sb[:, :, :, Dh : Dh + 1], 1.0)
            hT_sb = attnp.tile([128, KD, S], BF, tag="hT")

            # --- qT / kT projections (T layout) ---
            for wname, dst, bias_c, scale in (
                ("wq_bf", qT_sb, bq_c, 0.125),
                ("wk_bf", kT_sb, bk_c, None),
            ):
                w_t = []
                for k in range(KD):
                    wt = attnp.tile([128, D], BF, tag="w6", bufs=8, name=f"{wname}{k}")
                    nc.sync.dma_start(out=wt, in_=io[wname].rearrange("(c p) n -> c p n", p=128)[k])
                    w_t.append(wt)
                for m in range(KD):
                    ps = psum.tile([128, 512], F32, tag="acc", bufs=4, name="ps_qk")
                    for k in range(KD):
                        nc.tensor.matmul(
                            ps, w_t[k][:, 128 * m : 128 * (m + 1)], xT_sb[:, k, :],
                            start=(k == 0), stop=(k == KD - 1),
                        )
                    if scale is not None:
                        nc.vector.tensor_scalar(
                            out=dst[:, m, :], in0=ps,
                            scalar1=bias_c[:, m : m + 1], scalar2=scale,
                            op0=OP.add, op1=OP.mult,
                        )
                    else:
                        nc.vector.tensor_scalar_add(
                            out=dst[:, m, :], in0=ps, scalar1=bias_c[:, m : m + 1]
                        )

            # --- v projection (natural layout, bias via K-augmentation) ---
            wv_t = []
            for k in range(KD):
                wt = attnp.tile([128, D], BF, tag="w6", bufs=8, name=f"wv{k}")
                nc.sync.dma_start(out=wt, in_=io["wv_bf"].rearrange("(c p) n -> c p n", p=128)[k])
                wv_t.append(wt)
            for c in range(MS):
                for n in range(NT):
                    ps = psum.tile([128, ND], F32, tag="acc", bufs=4, name="ps_v")
                    for k in range(KD):
                        nc.tensor.matmul(
                            ps, xT_sb[:, k, 128 * c : 128 * (c + 1)],
                            wv_t[k][:, ND * n : ND * (n + 1)],
                            start=(k == 0), stop=False,
                        )
                    nc.tensor.matmul(
                        ps, ones_bf[:, 0:128], bv_r[:, ND * n : ND * (n + 1)],
                        start=False, stop=True,
                    )
                    nc.vector.tensor_copy(
                        out=v_sb[:, c, 6 * n : 6 * (n + 1), 0:Dh],
                        in_=ps.rearrange("p (h d) -> p h d", d=Dh),
                    )

            # --- per-head attention ---
            for h in range(H):
                th, off = h // 2, (h % 2) * 64
                qh = qT_sb[off : off + 64, th, :]
                kh = kT_sb[off : off + 64, th, :]
                exp_tiles = []
                for c in range(MS):
                    ps_sc = psum.tile([128, 512], F32, tag="acc", bufs=4, name="ps_sc")
                    nc.tensor.matmul(
                        ps_sc, kh[:, 128 * c : 128 * (c + 1)], qh, start=True, stop=True
                    )
                    tmp = attnp.tile([128, 512], F32, tag="exptmp", bufs=3, name="tmp")
                    nc.vector.tensor_tensor(out=tmp, in0=ps_sc, in1=gauss_sb[:, c, :], op=OP.add)
                    ex = attnp.tile([128, 512], BF, tag="exp", bufs=6, name="ex")
                    nc.scalar.activation(
                        out=ex, in_=tmp, func=AF.Exp, bias=maskb_c[:, c : c + 1], scale=1.0
                    )
                    exp_tiles.append(ex)
                ps_h = psum.tile([Dh + 1, 512], F32, tag="hT", bufs=2, name="ps_h")
                for c in range(MS):
                    nc.tensor.matmul(
                        ps_h, v_sb[:, c, h, :], exp_tiles[c],
                        start=(c == 0), stop=(c == MS - 1),
                    )
                rec = attnp.tile([1, 512], F32, tag="rec", bufs=2, name="rec")
                nc.vector.reciprocal(rec, ps_h[Dh : Dh + 1, :])
                ps_b = psum.tile([64, 512], F32, tag="bc", bufs=2, name="ps_b")
                nc.tensor.matmul(ps_b, ones64_f, rec, start=True, stop=True)
                bca = attnp.tile([64, 512], F32, tag="bca", bufs=2, name="bca")
                nc.vector.tensor_copy(bca, ps_b)
                nc.vector.tensor_tensor(
                    out=hT_sb[off : off + 64, th, :], in0=ps_h[0:64, :], in1=bca, op=OP.mult
                )

            # --- proj + residual + LN1 (+ h1 transpose) ---
            wp_t = []
            for k in range(KD):
                wt = attnp.tile([128, D], BF, tag="w6", bufs=8, name=f"wp{k}")
                nc.sync.dma_start(out=wt, in_=io["wproj_bf"].rearrange("(c p) n -> c p n", p=128)[k])
                wp_t.append(wt)
            for m in range(MS):
                pss = []
                for n in range(NT):
                    ps = psum.tile([128, ND], F32, tag="acc", bufs=4, name="ps_pr")
                    for k in range(KD):
                        nc.tensor.matmul(
                            ps, hT_sb[:, k, 128 * m : 128 * (m + 1)],
                            wp_t[k][:, ND * n : ND * (n + 1)],
                            start=(k == 0), stop=False,
                        )
                    nc.tensor.matmul(
                        ps, ones_bf[:, 0:128], bproj_r[:, ND * n : ND * (n + 1)],
                        start=False, stop=True,
                    )
                    pss.append(ps)
                row = glob.tile([128, D], F32, tag="rowtmp", bufs=3, name="row")
                for n in range(NT):
                    nc.vector.tensor_tensor(
                        out=row[:, ND * n : ND * (n + 1)], in0=pss[n],
                        in1=x_sb[:, m, ND * n : ND * (n + 1)], op=OP.add,
                    )
                _layernorm(nc, glob, row, g1b, be1b, eps_t, h1_sb[:, m, :])
                h1bf = glob.tile([128, D], BF, tag="h1bf", bufs=2, name="h1bf")
                nc.vector.tensor_copy(h1bf, h1_sb[:, m, :])
                for f in range(KD):
                    ps_t = psum.tile([128, 128], BF, tag="bc", bufs=2, name="ps_t")
                    nc.tensor.transpose(ps_t, h1bf[:, 128 * f : 128 * (f + 1)], ident_bf)
                    nc.vector.tensor_copy(out=h1T_sb[:, f, 128 * m : 128 * (m + 1)], in_=ps_t)

        # ================= FFN scope =================
        with tc.tile_pool(name="ffn", bufs=1) as ffnp:
            w1_t = []
            for k in range(KD):
                wt = ffnp.tile([128, Fd], BF, tag="w1", bufs=6, name=f"w1_{k}")
                nc.sync.dma_start(out=wt, in_=io["w1_bf"].rearrange("(c p) n -> c p n", p=128)[k])
                w1_t.append(wt)
            for fm in range(KF):
                ps = psum.tile([128, 512], F32, tag="acc", bufs=4, name="ps_f1")
                for k in range(KD):
                    nc.tensor.matmul(
                        ps, w1_t[k][:, 128 * fm : 128 * (fm + 1)], h1T_sb[:, k, :],
                        start=(k == 0), stop=(k == KD - 1),
                    )
                nc.scalar.activation(
                    out=ff1T_sb[:, fm, :], in_=ps, func=AF.Gelu,
                    bias=b1_c[:, fm : fm + 1], scale=1.0,
                )

            w2_t = []
            for k in range(KF):
                wt = ffnp.tile([128, D], BF, tag="w2", bufs=KF, name=f"w2_{k}")
                nc.sync.dma_start(out=wt, in_=io["w2_bf"].rearrange("(c p) n -> c p n", p=128)[k])
                w2_t.append(wt)
            for m in range(MS):
                pss = []
                for n in range(NT):
                    ps = psum.tile([128, ND], F32, tag="acc", bufs=4, name="ps_f2")
                    for k in range(KF):
                        nc.tensor.matmul(
                            ps, ff1T_sb[:, k, 128 * m : 128 * (m + 1)],
                            w2_t[k][:, ND * n : ND * (n + 1)],
                            start=(k == 0), stop=False,
                        )
                    nc.tensor.matmul(
                        ps, ones_bf[:, 0:128], b2_r[:, ND * n : ND * (n + 1)],
                        start=False, stop=True,
                    )
                    pss.append(ps)
                row = glob.tile([128, D], F32, tag="rowtmp", bufs=3, name="row2")
                for n in range(NT):
                    nc.vector.tensor_tensor(
                        out=row[:, ND * n : ND * (n + 1)], in0=pss[n],
                        in1=h1_sb[:, m, ND * n : ND * (n + 1)], op=OP.add,
                    )
                outrow = glob.tile([128, D], F32, tag="outrow", bufs=2, name="outrow")
                _layernorm(nc, glob, row, g2b, be2b, eps_t, outrow)
                nc.sync.dma_start(
                    out=io["out"][128 * m : 128 * (m + 1), :], in_=outrow
                )


def _layernorm(nc, pool, row, gamma_b, beta_b, eps_t, out_ap):
    st = pool.tile([128, 3, 6], F32, tag="st", bufs=4, name="st")
    for g in range(3):
        nc.vector.bn_stats(out=st[:, g, :], in_=row[:, 256 * g : 256 * (g + 1)])
    mv = pool.tile([128, 2], F32, tag="mv", bufs=4, name="mv")
    nc.vector.bn_aggr(out=mv, in_=st)
    sd = pool.tile([128, 1], F32, tag="sd", bufs=4, name="sd")
    nc.scalar.activation(out=sd, in_=mv[:, 1:2], func=AF.Sqrt, bias=eps_t, scale=1.0)
    rs = pool.tile([128, 1], F32, tag="rs", bufs=4, name="rs")
    nc.vector.reciprocal(rs, sd)
    t2 = pool.tile([128, D], F32, tag="rowtmp2", bufs=3, name="t2")
    nc.vector.scalar_tensor_tensor(
        out=t2, in0=row, scalar=mv[:, 0:1], in1=gamma_b, op0=OP.subtract, op1=OP.mult
    )
    nc.vector.scalar_tensor_tensor(
        out=out_ap, in0=t2, scalar=rs, in1=beta_b, op0=OP.mult, op1=OP.add
    )


_SPECS = [
    # (name, shape, dtype, per_core)
    ("x", [S, D], F32, True),
    ("xT_bf", [D, S], BF, True),
    ("maskbias", [S], F32, True),
    ("gaussT", [S, S], F32, False),
    ("wq_bf", [D, D], BF, False),
    ("wk_bf", [D, D], BF, False),
    ("wv_bf", [D, D], BF, False),
    ("wproj_bf", [D, D], BF, False),
    ("w1_bf", [D, Fd], BF, False),
    ("w2_bf", [Fd, D], BF, False),
    ("bq", [D], F32, False),
    ("bk", [D], F32, False),
    ("bv_bf", [D], BF, False),
    ("bproj_bf", [D], BF, False),
    ("b1", [Fd], F32, False),
    ("b2_bf", [D], BF, False),
    ("gamma1", [D], F32, False),
    ("beta1", [D], F32, False),
    ("gamma2", [D], F32, False),
    ("beta2", [D], F32, False),
]

_BUILT = {}


def _build():
    if "nc" in _BUILT:
        return _BUILT["nc"]
    nc = bacc.Bacc("TRN2", target_bir_lowering=False, debug=False,
                   enable_asserts=False, num_devices=NCORES)
    io = {}
    for name, shape, dt, _ in _SPECS:
        io[name] = nc.dram_tensor(name, shape, dt, kind="ExternalInput").ap()
    io["out"] = nc.dram_tensor("out", [S, D], F32, kind="ExternalOutput").ap()
    _trace(nc, io)
    nc.compile()
    _BUILT["nc"] = nc
    return nc


def _host_prep(inputs):
    bf = ml_dtypes.bfloat16
    f32 = np.float32
    x = np.asarray(inputs["x"], f32)
    mask = np.asarray(inputs["mask"])

    idx = np.arange(S, dtype=np.float64)
    dd = idx[None, :] - idx[:, None]
    sc = -0.5 * dd * dd
    sc -= sc.max(axis=-1, keepdims=True)
    e = np.exp(sc)
    gauss = (e / e.sum(axis=-1, keepdims=True)).astype(f32)  # [i=s, j=t]
    gaussT = np.ascontiguousarray(gauss.T)

    shared = {
        "gaussT": gaussT,
        "wq_bf": np.asarray(inputs["wq"], f32).astype(bf),
        "wk_bf": np.asarray(inputs["wk"], f32).astype(bf),
        "wv_bf": np.asarray(inputs["wv"], f32).astype(bf),
        "wproj_bf": np.asarray(inputs["w_proj"], f32).astype(bf),
        "w1_bf": np.asarray(inputs["w1"], f32).astype(bf),
        "w2_bf": np.asarray(inputs["w2"], f32).astype(bf),
        "bq": np.asarray(inputs["bq"], f32),
        "bk": np.asarray(inputs["bk"], f32),
        "bv_bf": np.asarray(inputs["bv"], f32).astype(bf),
        "bproj_bf": np.asarray(inputs["b_proj"], f32).astype(bf),
        "b1": np.asarray(inputs["b1"], f32),
        "b2_bf": np.asarray(inputs["b2"], f32).astype(bf),
        "gamma1": np.asarray(inputs["gamma1"], f32),
        "beta1": np.asarray(inputs["beta1"], f32),
        "gamma2": np.asarray(inputs["gamma2"], f32),
        "beta2": np.asarray(inputs["beta2"], f32),
    }
    in_maps = []
    for b in range(NCORES):
        m = dict(shared)
        m["x"] = np.ascontiguousarray(x[b])
        m["xT_bf"] = np.ascontiguousarray(x[b].T).astype(bf)
        m["maskbias"] = (MASK_NEG * (1.0 - mask[b].astype(f32))).astype(f32)
        in_maps.append(m)
    return in_maps


def _run(inputs, trace=False, trace_cores=None):
    nc = _build()
    in_maps = _host_prep(inputs)
    res = bass_utils.run_bass_kernel_spmd(
        nc, in_maps, core_ids=list(range(NCORES)), trace=trace,
        trace_cores=trace_cores,
    )
    out = np.stack([np.asarray(res.results[b]["out"]) for b in range(NCORES)])
    return out.astype(np.float32), res


def kernel(**inputs):
    return _run(inputs)[0]
